# revision 35
# baseline (speedup 1.0000x reference)
"""Trainium2 Bass kernel for CrossAttention (B=4, L=S=2048, DIM=1024, H=16, hd=64).

Sharding: data-parallel over (batch, L-half): core c handles batch c//2,
query rows [(c%2)*1024, (c%2+1)*1024).  Each core computes the QKV
projections for its slice (K/V duplicated within a batch pair), per-head
RMSNorm, masked softmax attention, and the output projection.

Device layout is feature-major ("transposed"): activations live as
[dim, tokens] so every matmul contraction dim is on SBUF partitions with
no on-device transposes.  The host pre-transposes q/kv and casts to bf16.

Softmax: after RMS norm |score| <= 8, so no running max is needed.  exp
runs on ACT with a fused per-partition scale (k-norm rsqrt / 8) and bias
(padding mask, -1e5 -> exp == 0).  The denominator comes from a 65th
"ones" column appended to V; o^T is normalized in two 4-pair groups with
one batched DVE reciprocal each, a K=8 indicator matmul broadcast and an
in-place DVE multiply, staged so nothing ever waits on reciprocal latency.

Perf notes: every DMA trigger costs ~0.65us serialized on the sync queue,
so transfers are batched into single multi-dim-AP DMA instructions
(weights 1 each, constants 2 blobs, gather stage/readback 1-2 each,
denominators 1 per pair, output 2).  Score matmuls for a head pair run
via tile_position row packing; PV matmuls are emitted one iteration late
so the PE queue head never waits on ACT; the attention phase is bound by
the ACT engine's exp throughput and runs it gap-free.  The output
projection is k-outer over 8 PSUM accumulators, interleaved with the
group-1 normalization so the PE never idles into the tail.
"""

import sys

if "/opt/trn_rl_repo" not in sys.path:
    sys.path.insert(0, "/opt/trn_rl_repo")

import numpy as np
import ml_dtypes

import concourse.bass as bass
import concourse.bacc as bacc
import concourse.tile as tile
from concourse import mybir
from concourse.bass_utils import run_bass_kernel_spmd

BF16 = ml_dtypes.bfloat16

B, L, S, DIM = 4, 2048, 2048, 1024
H, HD = 16, 64
N_CORES = 8
LC = L // 2          # query rows per core
KC = DIM // 128      # 128-partition chunks of DIM
EPS = 1e-5
MASK_BIAS = -1.0e5   # exp(-1e5) == 0 in fp32

TRACE = False        # set by test.py for profiling
LAST_RESULT = {}     # exec_time_ns etc. for test.py

_CACHE = {}


def _build(n_sc):
    """Build the SPMD Bass program; n_sc = number of 128-wide kv chunks."""
    fp32 = mybir.dt.float32
    bf16 = mybir.dt.bfloat16
    AF = mybir.ActivationFunctionType

    nc = bacc.Bacc("TRN2", target_bir_lowering=False, debug=False,
                   num_devices=N_CORES)

    n_half = (n_sc + 1) // 2         # kv chunks computed locally per core
    W = n_half * 128                 # local kv width
    qT_d = nc.dram_tensor("qT", [DIM, LC], bf16, kind="ExternalInput")
    kvT_d = nc.dram_tensor("kvT", [DIM, W], bf16, kind="ExternalInput")
    wq_d = nc.dram_tensor("wq", [DIM, DIM], bf16, kind="ExternalInput")
    wk_d = nc.dram_tensor("wk", [DIM, DIM], bf16, kind="ExternalInput")
    wv_d = nc.dram_tensor("wv", [DIM, DIM], bf16, kind="ExternalInput")
    wo_d = nc.dram_tensor("wo", [DIM, DIM], bf16, kind="ExternalInput")
    # packed constant blobs: cf32 = mask(16) | qw(1) | kw(1) | eye16(16)
    # cbf  = ind(8x16) | ind2(8x128 on rows 0-15) | ind2p8(4x128 rows 0-7)
    cf32_d = nc.dram_tensor("cf32", [128, 34], fp32, kind="ExternalInput")
    cbf_d = nc.dram_tensor("cbf", [128, 128 + 1024 + 512], bf16,
                           kind="ExternalInput")
    out_d = nc.dram_tensor("out", [LC, DIM], fp32, kind="ExternalOutput")

    n_hg = (W + 511) // 512          # 512-wide groups over the local half
    VA_W = H * 65

    with tile.TileContext(nc) as tc:
        with (
            tc.tile_pool(name="wp", bufs=3) as wp,       # wk wv wq (+wo reuse)
            tc.tile_pool(name="kvb", bufs=1) as kvb,     # kvT big
            tc.tile_pool(name="qtp", bufs=KC) as qtp,    # qT, later oT
            tc.tile_pool(name="sqp", bufs=6) as sqp,     # square/exp tiles
            tc.tile_pool(name="qhp", bufs=KC) as qhp,    # qhT
            tc.tile_pool(name="khb", bufs=1) as khb,     # khh + kh big
            tc.tile_pool(name="vb", bufs=1) as vb,       # va local + full big
            tc.tile_pool(name="sp", bufs=1) as sp,       # constants etc.
            tc.tile_pool(name="tp", bufs=4) as tp,       # f32 temps
            tc.tile_pool(name="dp", bufs=1, space="DRAM") as dp,   # blobs
            tc.tile_pool(name="pa", bufs=2, space="PSUM") as pa,   # scores
            tc.tile_pool(name="po", bufs=4, space="PSUM") as po,   # pv etc
        ):
            # ---------------- inputs (batched DMAs) ----------------
            # order follows first use: K-proj needs wk+kvT first.
            wk_b = wp.tile([128, KC * DIM], bf16, name="wk", tag="w")
            nc.sync.dma_start(
                out=wk_b.rearrange("p (k c) -> p k c", k=KC),
                in_=wk_d.rearrange("(k p) c -> p k c", p=128))
            wk_sb = [wk_b[:, k * DIM:(k + 1) * DIM] for k in range(KC)]

            kv_b = kvb.tile([128, KC * W], bf16, name="kvt")
            nc.sync.dma_start(
                out=kv_b.rearrange("p (k c) -> p k c", k=KC),
                in_=kvT_d.rearrange("(k p) c -> p k c", p=128))
            kvt_sb = [kv_b[:, k * W:(k + 1) * W] for k in range(KC)]

            cf32_sb = sp.tile([128, 34], fp32, name="cf32")
            nc.sync.dma_start(out=cf32_sb, in_=cf32_d[:, :])
            mask_sb = cf32_sb[:, 0:16]
            qw_sb = cf32_sb[:, 16:17]
            kw_sb = cf32_sb[:, 17:18]
            eye_sb = cf32_sb[0:16, 18:34]
            cbf_sb = sp.tile([128, 1664], bf16, name="cbf")
            nc.sync.dma_start(out=cbf_sb, in_=cbf_d[:, :])
            ind_sb = [cbf_sb[:, k * 16:(k + 1) * 16] for k in range(KC)]
            ind2_sb = [cbf_sb[0:16, 128 + m * 128:128 + (m + 1) * 128]
                       for m in range(KC)]
            ind2p8_sb = [cbf_sb[0:8, 1152 + r * 128:1152 + (r + 1) * 128]
                         for r in range(4)]
            epsq_sb = sp.tile([16, 1], fp32, name="epsq")
            nc.vector.memset(epsq_sb, EPS)
            epsk_sb = sp.tile([16, 1], fp32, name="epsk")
            nc.vector.memset(epsk_sb, 64.0 * EPS)

            wv_b = wp.tile([128, KC * DIM], bf16, name="wv", tag="w")
            nc.sync.dma_start(
                out=wv_b.rearrange("p (k c) -> p k c", k=KC),
                in_=wv_d.rearrange("(k p) c -> p k c", p=128))
            wv_sb = [wv_b[:, k * DIM:(k + 1) * DIM] for k in range(KC)]

            qt_sb = []
            for k in range(KC):
                qt = qtp.tile([128, LC], bf16, name=f"qt{k}", tag="qt")
                nc.sync.dma_start(out=qt, in_=qT_d[k * 128:(k + 1) * 128, :])
                qt_sb.append(qt)

            wq_b = wp.tile([128, KC * DIM], bf16, name="wq", tag="w")
            nc.sync.dma_start(
                out=wq_b.rearrange("p (k c) -> p k c", k=KC),
                in_=wq_d.rearrange("(k p) c -> p k c", p=128))
            wq_sb = [wq_b[:, k * DIM:(k + 1) * DIM] for k in range(KC)]

            # -------- K projection + skT on the LOCAL kv half --------
            khh_b = khb.tile([128, KC * W], bf16, name="khh", tag="khh")
            khh_sb = [khh_b[:, m * W:(m + 1) * W] for m in range(KC)]
            skl_b = sp.tile([128, n_half * 16], fp32, name="skl")
            for sg in range(n_hg):
                wdt = min(512, W - sg * 512)
                ssk = po.tile([16, 512], fp32, name="ssk", tag="po")
                pend = None
                for m in range(KC):
                    ps = pa.tile([128, 1024], fp32, name="proj_ps", tag="pa")
                    for k in range(KC):
                        nc.tensor.matmul(
                            ps[:, :wdt],
                            lhsT=wk_sb[k][:, m * 128:(m + 1) * 128],
                            rhs=kvt_sb[k][:, sg * 512:sg * 512 + wdt],
                            start=(k == 0), stop=(k == KC - 1))
                    if pend is not None:
                        pm, pq = pend
                        nc.tensor.matmul(
                            ssk[:, :wdt], lhsT=ind_sb[pm], rhs=pq[:, :wdt],
                            start=(pm == 0), stop=False)
                    nc.vector.tensor_scalar_mul(
                        khh_sb[m][:, sg * 512:sg * 512 + wdt], ps[:, :wdt],
                        kw_sb)
                    ksq = sqp.tile([128, 1024], bf16, name="sqt", tag="sq",
                                   bufs=6)
                    nc.scalar.activation(ksq[:, :wdt], ps[:, :wdt], AF.Square)
                    pend = (m, ksq)
                pm, pq = pend
                nc.tensor.matmul(ssk[:, :wdt], lhsT=ind_sb[pm],
                                 rhs=pq[:, :wdt], start=False, stop=True)
                # 8*sqrt(mean+eps) = sqrt(sumsq + 64 eps); recip -> sk/8
                skr = tp.tile([16, 512], fp32, name="skr", tag="small16",
                              bufs=2)
                nc.scalar.activation(skr[:, :wdt], ssk[:, :wdt], AF.Sqrt,
                                     scale=1.0, bias=epsk_sb)
                for t in range(wdt // 128):
                    tpp = po.tile([128, 16], fp32, name="sktp", tag="po")
                    nc.tensor.transpose(tpp, skr[:, t * 128:(t + 1) * 128],
                                        eye_sb)
                    i = sg * 4 + t
                    nc.vector.reciprocal(out=skl_b[:, i * 16:(i + 1) * 16],
                                         in_=tpp)

            # -------- AllGather 1: kh half (overlaps the V projection) ----
            TOTA = KC * 128 * W
            TOTB = n_half * 128 * (VA_W + 32)
            off_sk = n_half * 128 * VA_W
            groups = [[2 * x, 2 * x + 1] for x in range(N_CORES // 2)]
            blobA_loc = dp.tile([TOTA], bf16, name="blobA_loc")
            blobA_g = dp.tile([2 * TOTA], bf16, name="blobA_g")
            nc.sync.dma_start(
                out=blobA_loc.rearrange("(m p x) -> p m x", m=KC, p=128),
                in_=khh_b.rearrange("p (m x) -> p m x", m=KC))
            nc.gpsimd.collective_compute(
                "AllGather", mybir.AluOpType.bypass, replica_groups=groups,
                ins=[blobA_loc.opt()], outs=[blobA_g.opt()])

            # -------- V projection on the LOCAL kv half (ones-augmented) ----
            val_b = vb.tile([128, n_half * VA_W], bf16, name="va_loc",
                            tag="val")
            ones_ap = bass.AP(
                tensor=val_b.tensor, offset=val_b.offset + 64,
                ap=[list(val_b.ap[0]), [VA_W, n_half], [65, H], [1, 1]])
            nc.vector.memset(ones_ap, 1.0)
            for i in range(n_half):
                ps = pa.tile([128, 1024], fp32, name="proj_ps", tag="pa")
                for k in range(KC):          # k outer: one LDW serves both jn
                    for jn in range(2):
                        nc.tensor.matmul(
                            ps[:, jn * 512:(jn + 1) * 512],
                            lhsT=kvt_sb[k][:, i * 128:(i + 1) * 128],
                            rhs=wv_sb[k][:, jn * 512:(jn + 1) * 512],
                            start=(k == 0), stop=(k == KC - 1))
                for jn in range(2):
                    dst = bass.AP(
                        tensor=val_b.tensor,
                        offset=val_b.offset + i * VA_W + 65 * 8 * jn,
                        ap=[list(val_b.ap[0]), [65, 8], [1, 64]])
                    nc.vector.tensor_copy(
                        dst, ps[:, jn * 512:(jn + 1) * 512]
                        .rearrange("p (h d) -> p h d", h=8))

            # -------- AllGather 2: va + sk (overlaps the Q projection) ----
            blobB_loc = dp.tile([TOTB], bf16, name="blobB_loc")
            blobB_g = dp.tile([2 * TOTB], bf16, name="blobB_g")
            nc.sync.dma_start(
                out=blobB_loc[0:off_sk]
                .rearrange("(l p x) -> p l x", l=n_half, p=128),
                in_=val_b.rearrange("p (l x) -> p l x", l=n_half))
            nc.sync.dma_start(
                out=blobB_loc[off_sk:TOTB]
                .rearrange("(l p x) -> p l x", l=n_half, p=128),
                in_=skl_b.bitcast(bf16).rearrange("p (l x) -> p l x",
                                                  l=n_half))
            nc.gpsimd.collective_compute(
                "AllGather", mybir.AluOpType.bypass, replica_groups=groups,
                ins=[blobB_loc.opt()], outs=[blobB_g.opt()])

            # ---------------- Q projection + q RMS stats ----------------
            # sumsq matmuls are emitted one (m, j) step late so the PE
            # queue head never blocks on the ACT Square.
            qh_sb = [qhp.tile([128, LC], bf16, name=f"qh{m}", tag="qh")
                     for m in range(KC)]
            sumsq_q = [po.tile([16, 512], fp32, name=f"ssq{j}", tag="po")
                       for j in range(2)]
            pend = None                      # (m, qsq_tile)
            for m in range(KC):
                ps = pa.tile([128, 1024], fp32, name="proj_ps", tag="pa")
                for k in range(KC):          # k outer: one LDW serves both j
                    for j in range(2):
                        nc.tensor.matmul(
                            ps[:, j * 512:(j + 1) * 512],
                            lhsT=wq_sb[k][:, m * 128:(m + 1) * 128],
                            rhs=qt_sb[k][:, j * 512:(j + 1) * 512],
                            start=(k == 0), stop=(k == KC - 1))
                if pend is not None:
                    pm, pq = pend
                    for j in range(2):
                        nc.tensor.matmul(
                            sumsq_q[j][:, :], lhsT=ind_sb[pm],
                            rhs=pq[:, j * 512:(j + 1) * 512],
                            start=(pm == 0), stop=(pm == KC - 1))
                nc.vector.tensor_scalar_mul(qh_sb[m][:, :], ps[:, :], qw_sb)
                qsq = sqp.tile([128, 1024], bf16, name="sqt", tag="sq",
                               bufs=6)
                nc.scalar.activation(qsq, ps, AF.Square)
                pend = (m, qsq)
            pm, pq = pend
            for j in range(2):
                nc.tensor.matmul(sumsq_q[j][:, :], lhsT=ind_sb[pm],
                                 rhs=pq[:, j * 512:(j + 1) * 512],
                                 start=False, stop=True)
            # sq = 1/sqrt(mean + eps), broadcast onto qhT via ind2 matmuls
            sq_sb = []
            for j in range(2):
                sqr = tp.tile([16, 512], fp32, name=f"sqr{j}", tag="small16",
                              bufs=2)
                nc.scalar.activation(sqr, sumsq_q[j][:, :], AF.Sqrt,
                                     scale=1.0 / HD, bias=epsq_sb)
                sqv = tp.tile([16, 512], bf16, name=f"sqv{j}",
                              tag="small16b", bufs=2)
                with nc.allow_low_precision(
                        reason="rsqrt scale applied to bf16 qh"):
                    nc.vector.reciprocal(out=sqv, in_=sqr)
                sq_sb.append(sqv)
            for m in range(KC):
                for j in range(2):
                    bc = po.tile([128, 512], fp32, name="qbc", tag="po")
                    nc.tensor.matmul(bc, lhsT=ind2_sb[m], rhs=sq_sb[j],
                                     start=True, stop=True)
                    nc.vector.tensor_mul(
                        qh_sb[m][:, j * 512:(j + 1) * 512],
                        qh_sb[m][:, j * 512:(j + 1) * 512], bc)

            # wo load: issued here so the DMA overlaps the attention phase
            # (the big tile reuses wk's pool slot, free once K-proj is done).
            wo_b = wp.tile([128, KC * DIM], bf16, name="wo", tag="w")
            nc.sync.dma_start(
                out=wo_b.rearrange("p (k c) -> p k c", k=KC),
                in_=wo_d.rearrange("(k p) c -> p k c", p=128))
            wo_sb = [wo_b[:, k * DIM:(k + 1) * DIM] for k in range(KC)]

            # -------- readback into canonical full-S tiles --------
            kh_b = khb.tile([128, KC * 2 * W], bf16, name="kh", tag="kh")
            kh_sb = [kh_b[:, m * 2 * W:(m + 1) * 2 * W] for m in range(KC)]
            for r in range(2):
                dst = bass.AP(tensor=kh_b.tensor,
                              offset=kh_b.offset + r * W,
                              ap=[list(kh_b.ap[0]), [2 * W, KC], [1, W]])
                nc.sync.dma_start(
                    out=dst,
                    in_=blobA_g[r * TOTA:(r + 1) * TOTA]
                    .rearrange("(m p x) -> p m x", m=KC, p=128))
            va_b = vb.tile([128, n_sc * VA_W], bf16, name="va", tag="va")
            va_sb = [va_b[:, i * VA_W:(i + 1) * VA_W] for i in range(n_sc)]
            skT_b = sp.tile([128, n_sc * 16], fp32, name="skT")
            for r in range(2):
                cnt = n_half if r == 0 else n_sc - n_half
                nc.sync.dma_start(
                    out=va_b[:, r * n_half * VA_W:
                             (r * n_half + cnt) * VA_W]
                    .rearrange("p (l x) -> p l x", l=cnt),
                    in_=blobB_g[r * TOTB:r * TOTB + cnt * 128 * VA_W]
                    .rearrange("(l p x) -> p l x", l=cnt, p=128))
                nc.sync.dma_start(
                    out=skT_b.bitcast(bf16)[:, r * n_half * 32:
                                            (r * n_half + cnt) * 32]
                    .rearrange("p (l x) -> p l x", l=cnt),
                    in_=blobB_g[r * TOTB + off_sk:
                                r * TOTB + off_sk + cnt * 128 * 32]
                    .rearrange("(l p x) -> p l x", l=cnt, p=128))

            # ---------------- attention (head pairs, pipelined) ----------
            # oT is normalized in two 4-pair groups: per pair the 4 pv
            # denominator rows (partition 64) are cast into one staging
            # tile and moved by ONE DMA into the packed group tile; one
            # batched DVE reciprocal per group (hidden behind attention /
            # the O-proj k<4 accumulation), then per (pair, j) one K=8
            # indicator matmul broadcast + one in-place DVE multiply.
            oT_sb = [qtp.tile([128, LC], bf16, name=f"oT{m}", tag="qt")
                     for m in range(KC)]
            den8 = [sp.tile([8, LC], bf16, name=f"den8_{g}")
                    for g in range(2)]
            rec8 = [sp.tile([8, LC], bf16, name=f"rec8_{g}")
                    for g in range(2)]

            def norm_apply(g):
                # broadcast 1/den and rescale oT for pairs 4g..4g+3
                for r in range(4):
                    pp = 4 * g + r
                    for j in range(2):
                        dbc = po.tile([128, 512], fp32, name="dbc",
                                      tag="po")
                        nc.tensor.matmul(
                            dbc, lhsT=ind2p8_sb[r],
                            rhs=rec8[g][:, j * 512:(j + 1) * 512],
                            start=True, stop=True)
                        nc.vector.tensor_mul(
                            oT_sb[pp][:, j * 512:(j + 1) * 512],
                            oT_sb[pp][:, j * 512:(j + 1) * 512], dbc)

            for p in range(KC):              # head pair (2p, 2p+1)
                hA, hB = 2 * p, 2 * p + 1
                pv = [po.tile([128, 512], fp32, name=f"pv{x}", tag="po")
                      for x in range(4)]     # A0 A1 B0 B1
                pending = None               # (exA, exB, first)
                for i in range(n_sc):
                    scA = pa.tile([128, 1024], fp32, name="scA", tag="pa")
                    scB = pa.tile([128, 1024], fp32, name="scB", tag="pa")
                    for j in range(2):
                        nc.tensor.matmul(
                            scA[:, j * 512:(j + 1) * 512],
                            lhsT=kh_sb[p][0:64, i * 128:(i + 1) * 128],
                            rhs=qh_sb[p][0:64, j * 512:(j + 1) * 512],
                            start=True, stop=True, tile_position=(0, 0))
                        nc.tensor.matmul(
                            scB[:, j * 512:(j + 1) * 512],
                            lhsT=kh_sb[p][64:128, i * 128:(i + 1) * 128],
                            rhs=qh_sb[p][64:128, j * 512:(j + 1) * 512],
                            start=True, stop=True, tile_position=(64, 0))
                    if pending is not None:
                        exA, exB, first = pending
                        for j in range(2):
                            nc.tensor.matmul(
                                pv[j][:65, :],
                                lhsT=va_sb[i - 1][:, hA * 65:(hA + 1) * 65],
                                rhs=exA[:, j * 512:(j + 1) * 512],
                                start=first, stop=False)
                        for j in range(2):
                            nc.tensor.matmul(
                                pv[2 + j][:65, :],
                                lhsT=va_sb[i - 1][:, hB * 65:(hB + 1) * 65],
                                rhs=exB[:, j * 512:(j + 1) * 512],
                                start=first, stop=False)
                    exA = sqp.tile([128, 1024], bf16, name="exA", tag="sq",
                                   bufs=6)
                    exB = sqp.tile([128, 1024], bf16, name="exB", tag="sq",
                                   bufs=6)
                    nc.scalar.activation(
                        exA, scA, AF.Exp,
                        scale=skT_b[:, i * 16 + hA:i * 16 + hA + 1],
                        bias=mask_sb[:, i:i + 1])
                    nc.scalar.activation(
                        exB, scB, AF.Exp,
                        scale=skT_b[:, i * 16 + hB:i * 16 + hB + 1],
                        bias=mask_sb[:, i:i + 1])
                    pending = (exA, exB, i == 0)
                exA, exB, first = pending
                for j in range(2):
                    nc.tensor.matmul(
                        pv[j][:65, :],
                        lhsT=va_sb[n_sc - 1][:, hA * 65:(hA + 1) * 65],
                        rhs=exA[:, j * 512:(j + 1) * 512],
                        start=first, stop=True)
                for j in range(2):
                    nc.tensor.matmul(
                        pv[2 + j][:65, :],
                        lhsT=va_sb[n_sc - 1][:, hB * 65:(hB + 1) * 65],
                        rhs=exB[:, j * 512:(j + 1) * 512],
                        start=first, stop=True)
                # stash unnormalized o^T; cast the 4 den rows into one
                # staging tile and move them with ONE DMA into the packed
                # [8, LC] group tile (rows 2r, 2r+1).
                g, r = p // 4, p % 4
                dstage = tp.tile([128, 2048], bf16, name="dstage",
                                 tag="rec", bufs=2)
                for x, (hh, j) in enumerate(((hA, 0), (hA, 1),
                                             (hB, 0), (hB, 1))):
                    poff = (hh % 2) * 64
                    nc.vector.tensor_copy(
                        oT_sb[p][poff:poff + 64, j * 512:(j + 1) * 512],
                        pv[x][0:64, :])
                    nc.vector.tensor_copy(
                        dstage[64:65, x * 512:(x + 1) * 512],
                        pv[x][64:65, :])
                nc.sync.dma_start(out=den8[g][2 * r:2 * r + 2, :],
                                  in_=dstage[64:65, :])
                if p == 4:
                    # group-0 reciprocal on DVE, overlapped with pair 5
                    with nc.allow_low_precision(
                            reason="1/den scale applied to bf16 oT"):
                        nc.vector.reciprocal(out=rec8[0], in_=den8[0])
                if p == 5:
                    norm_apply(0)

            # ---------------- output projection ----------------
            # k-outer with 8 parallel PSUM accumulators.  The group-1
            # normalization is interleaved: a few k<3 groups of the lc0-3
            # accumulators run while the reciprocal drains, then the
            # broadcast+rescale, then everything else.  PSUM->SBUF copies
            # alternate scalar (idle post-attention) and vector; the out
            # DMA is one instruction per 512-col half.
            with nc.allow_low_precision(
                    reason="1/den scale applied to bf16 oT"):
                nc.vector.reciprocal(out=rec8[1], in_=den8[1])
            oacc = []
            for t in range(2):
                pt = pa.tile([128, 1024], fp32, name=f"oacc{t}", tag="pa")
                oacc.append(pt[:, 0:512])
                oacc.append(pt[:, 512:1024])

            def oproj_mm(jn, k, lc):
                nc.tensor.matmul(
                    oacc[lc], lhsT=oT_sb[k][:, lc * 128:(lc + 1) * 128],
                    rhs=wo_sb[k][:, jn * 512:(jn + 1) * 512],
                    start=(k == 0), stop=(k == KC - 1))

            for k in range(3):               # covers the reciprocal latency
                for lc in range(4):
                    oproj_mm(0, k, lc)
            norm_apply(1)
            for k in range(3, KC):
                for lc in range(4):
                    oproj_mm(0, k, lc)
            for t in range(4):
                oacc.append(po.tile([128, 512], fp32, name=f"oaccb{t}",
                                    tag="po"))
            for k in range(KC):
                for lc in range(4, KC):
                    oproj_mm(0, k, lc)
            osb = tp.tile([128, KC * 512], fp32, name="osb", tag="osb",
                          bufs=1)
            for jn in range(2):
                if jn == 1:
                    for k in range(KC):
                        for lc in range(KC):
                            oproj_mm(1, k, lc)
                for lc in range(KC):
                    dst = osb[:, lc * 512:(lc + 1) * 512]
                    if lc % 2 == 0:
                        nc.scalar.copy(out=dst, in_=oacc[lc])
                    else:
                        nc.vector.tensor_copy(dst, oacc[lc])
                nc.sync.dma_start(
                    out=out_d[:, jn * 512:(jn + 1) * 512]
                    .rearrange("(l p) c -> p l c", p=128),
                    in_=osb.rearrange("p (l c) -> p l c", l=KC))
    nc.compile()
    return nc


def kernel(**inputs):
    q = np.asarray(inputs["q"], dtype=np.float32)
    kv = np.asarray(inputs["kv"], dtype=np.float32)
    seqlens = np.asarray(inputs["x_seqlens"], dtype=np.int32)
    Wq = np.asarray(inputs["Wq"], dtype=np.float32)
    Wk = np.asarray(inputs["Wk"], dtype=np.float32)
    Wv = np.asarray(inputs["Wv"], dtype=np.float32)
    Wo = np.asarray(inputs["Wo"], dtype=np.float32)
    qnw = np.asarray(inputs["q_norm_w"], dtype=np.float32)
    knw = np.asarray(inputs["k_norm_w"], dtype=np.float32)

    n_sc = max(1, int(-(-int(seqlens.max()) // 128)))
    if n_sc not in _CACHE:
        _CACHE[n_sc] = _build(n_sc)
    nc = _CACHE[n_sc]

    wq_b = np.ascontiguousarray(Wq).astype(BF16)
    wk_b = np.ascontiguousarray(Wk).astype(BF16)
    wv_b = np.ascontiguousarray(Wv).astype(BF16)
    wo_b = np.ascontiguousarray(Wo).astype(BF16)
    ind = np.zeros((KC, 128, 16), np.float32)
    ind2 = np.zeros((KC, 16, 128), np.float32)
    p = np.arange(128)
    for c in range(KC):
        ind[c, p, 2 * c + p // 64] = 1.0
        ind2[c, 2 * c + p // 64, p] = 1.0
    cf32 = np.zeros((128, 34), np.float32)
    cf32[:, 16] = np.tile(qnw, 2)
    cf32[:, 17] = np.tile(knw, 2)
    cf32[0:16, 18:34] = np.eye(16, dtype=np.float32)
    cbf = np.zeros((128, 1664), np.float32)
    for k in range(KC):
        cbf[:, k * 16:(k + 1) * 16] = ind[k]
    for m in range(KC):
        cbf[0:16, 128 + m * 128:128 + (m + 1) * 128] = ind2[m]
    for r in range(4):
        cbf[2 * r, 1152 + r * 128:1152 + r * 128 + 64] = 1.0
        cbf[2 * r + 1, 1152 + r * 128 + 64:1152 + (r + 1) * 128] = 1.0
    cbf = cbf.astype(BF16)

    in_maps = []
    for c in range(N_CORES):
        b, half = c // 2, c % 2
        qT = np.ascontiguousarray(
            q[b, half * LC:(half + 1) * LC, :].T).astype(BF16)
        n_half = (n_sc + 1) // 2
        Wl = n_half * 128
        kvT = np.ascontiguousarray(
            kv[b].T[:, half * Wl:(half + 1) * Wl]).astype(BF16)
        sl = int(seqlens[b])
        mask = np.where(np.arange(S) < sl, 0.0, MASK_BIAS).astype(np.float32)
        cfc = cf32.copy()
        cfc[:, 0:16] = np.ascontiguousarray(mask.reshape(16, 128).T)
        in_maps.append({
            "qT": qT, "kvT": kvT, "wq": wq_b, "wk": wk_b, "wv": wv_b,
            "wo": wo_b, "cf32": cfc, "cbf": cbf,
        })

    res = run_bass_kernel_spmd(nc, in_maps, list(range(N_CORES)),
                               trace=TRACE)
    LAST_RESULT["exec_time_ns"] = res.exec_time_ns
    LAST_RESULT["profile"] = res.profile_json

    out = np.empty((B, L, DIM), np.float32)
    for c in range(N_CORES):
        b, half = c // 2, c % 2
        out[b, half * LC:(half + 1) * LC, :] = res.results[c]["out"]
    return out


# revision 44
# speedup vs baseline: 1.0230x; 1.0230x over previous
"""Trainium2 Bass kernel for CrossAttention (B=4, L=S=2048, DIM=1024, H=16, hd=64).

Sharding: data-parallel over (batch, L-half): core c handles batch c//2,
query rows [(c%2)*1024, (c%2+1)*1024).  Each core computes the QKV
projections for its slice (K/V duplicated within a batch pair), per-head
RMSNorm, masked softmax attention, and the output projection.

Device layout is feature-major ("transposed"): activations live as
[dim, tokens] so every matmul contraction dim is on SBUF partitions with
no on-device transposes.  The host pre-transposes q/kv and casts to bf16.

Softmax: after RMS norm |score| <= 8, so no running max is needed.  exp
runs on ACT with a fused per-partition scale (k-norm rsqrt / 8) and bias
(padding mask, -1e5 -> exp == 0).  The denominator comes from a 65th
"ones" column appended to V; o^T is normalized in two 4-pair groups with
one batched DVE reciprocal each, a K=8 indicator matmul broadcast and an
in-place DVE multiply, staged so nothing ever waits on reciprocal latency.

Perf notes: every DMA trigger costs ~0.65us serialized on the sync queue,
so transfers are batched into single multi-dim-AP DMA instructions
(weights 1 each, constants 2 blobs, gather stage/readback 1-2 each,
denominators 1 per pair, output 2).  Score matmuls for a head pair run
via tile_position row packing; PV matmuls are emitted one iteration late
so the PE queue head never waits on ACT; the attention phase is bound by
the ACT engine's exp throughput and runs it gap-free.  The output
projection is k-outer over 8 PSUM accumulators, interleaved with the
group-1 normalization so the PE never idles into the tail.
"""

import sys

if "/opt/trn_rl_repo" not in sys.path:
    sys.path.insert(0, "/opt/trn_rl_repo")

import numpy as np
import ml_dtypes

import concourse.bass as bass
import concourse.bacc as bacc
import concourse.tile as tile
from concourse import mybir
from concourse.bass_utils import run_bass_kernel_spmd

BF16 = ml_dtypes.bfloat16

B, L, S, DIM = 4, 2048, 2048, 1024
H, HD = 16, 64
N_CORES = 8
LC = L // 2          # query rows per core
KC = DIM // 128      # 128-partition chunks of DIM
EPS = 1e-5
MASK_BIAS = -1.0e5   # exp(-1e5) == 0 in fp32

TRACE = False        # set by test.py for profiling
LAST_RESULT = {}     # exec_time_ns etc. for test.py

_CACHE = {}


def _build(n_sc):
    """Build the SPMD Bass program; n_sc = number of 128-wide kv chunks."""
    fp32 = mybir.dt.float32
    bf16 = mybir.dt.bfloat16
    AF = mybir.ActivationFunctionType

    nc = bacc.Bacc("TRN2", target_bir_lowering=False, debug=False,
                   num_devices=N_CORES)

    n_half = (n_sc + 1) // 2         # kv chunks computed locally per core
    W = n_half * 128                 # local kv width
    qT_d = nc.dram_tensor("qT", [DIM, LC], bf16, kind="ExternalInput")
    kvT_d = nc.dram_tensor("kvT", [DIM, W], bf16, kind="ExternalInput")
    wq_d = nc.dram_tensor("wq", [DIM, DIM], bf16, kind="ExternalInput")
    wk_d = nc.dram_tensor("wk", [DIM, DIM], bf16, kind="ExternalInput")
    wv_d = nc.dram_tensor("wv", [DIM, DIM], bf16, kind="ExternalInput")
    wo_d = nc.dram_tensor("wo", [DIM, DIM], bf16, kind="ExternalInput")
    # packed constant blobs: cf32 = mask(16) | qw(1) | kw(1) | eye16(16)
    # cbf  = ind(8x16) | ind2(8x128 on rows 0-15) | ind2p8(4x128 rows 0-7)
    cf32_d = nc.dram_tensor("cf32", [128, 34], fp32, kind="ExternalInput")
    cbf_d = nc.dram_tensor("cbf", [128, 128 + 1024 + 512], bf16,
                           kind="ExternalInput")
    out_d = nc.dram_tensor("out", [LC, DIM], bf16, kind="ExternalOutput")

    n_hg = (W + 511) // 512          # 512-wide groups over the local half
    VA_W = H * 65

    with tile.TileContext(nc) as tc:
        with (
            tc.tile_pool(name="wp", bufs=3) as wp,       # wk wv wq (+wo reuse)
            tc.tile_pool(name="kvb", bufs=1) as kvb,     # kvT big
            tc.tile_pool(name="qtp", bufs=KC) as qtp,    # qT, later oT
            tc.tile_pool(name="sqp", bufs=6) as sqp,     # square/exp tiles
            tc.tile_pool(name="qhp", bufs=KC) as qhp,    # qhT
            tc.tile_pool(name="khb", bufs=1) as khb,     # khh + kh big
            tc.tile_pool(name="vb", bufs=1) as vb,       # va local + full big
            tc.tile_pool(name="sp", bufs=1) as sp,       # constants etc.
            tc.tile_pool(name="tp", bufs=4) as tp,       # f32 temps
            tc.tile_pool(name="dp", bufs=1, space="DRAM") as dp,   # blobs
            tc.tile_pool(name="pa", bufs=2, space="PSUM") as pa,   # scores
            tc.tile_pool(name="po", bufs=4, space="PSUM") as po,   # pv etc
        ):
            # ---------------- inputs ----------------
            # DMA instructions cost ~0.65us serialized on the sync queue
            # but each runs on its own ~22.5GB/s DMA engine: transfers are
            # split into 2-8 pieces to balance trigger cost vs parallelism.
            # Order follows first use: K-proj needs wk+kvT first.
            def dma_pieces(dst_big, src_2d, n_pieces, kc, width):
                # dst_big[:, k*width:(k+1)*width] <- src_2d rows k*128..
                per = kc // n_pieces
                for t in range(n_pieces):
                    k0 = t * per
                    nc.sync.dma_start(
                        out=dst_big[:, k0 * width:(k0 + per) * width]
                        .rearrange("p (k c) -> p k c", k=per),
                        in_=src_2d[k0 * 128:(k0 + per) * 128, :]
                        .rearrange("(k p) c -> p k c", p=128))

            wk_b = wp.tile([128, KC * DIM], bf16, name="wk", tag="w")
            dma_pieces(wk_b, wk_d, 8, KC, DIM)
            wk_sb = [wk_b[:, k * DIM:(k + 1) * DIM] for k in range(KC)]

            kv_b = kvb.tile([128, KC * W], bf16, name="kvt")
            dma_pieces(kv_b, kvT_d, 8, KC, W)
            kvt_sb = [kv_b[:, k * W:(k + 1) * W] for k in range(KC)]

            cf32_sb = sp.tile([128, 34], fp32, name="cf32")
            nc.sync.dma_start(out=cf32_sb, in_=cf32_d[:, :])
            mask_sb = cf32_sb[:, 0:16]
            qw_sb = cf32_sb[:, 16:17]
            kw_sb = cf32_sb[:, 17:18]
            eye_sb = cf32_sb[0:16, 18:34]
            cbf_sb = sp.tile([128, 1664], bf16, name="cbf")
            nc.sync.dma_start(out=cbf_sb, in_=cbf_d[:, :])
            ind_sb = [cbf_sb[:, k * 16:(k + 1) * 16] for k in range(KC)]
            ind2_sb = [cbf_sb[0:16, 128 + m * 128:128 + (m + 1) * 128]
                       for m in range(KC)]
            ind2p8_sb = [cbf_sb[0:8, 1152 + r * 128:1152 + (r + 1) * 128]
                         for r in range(4)]
            epsq_sb = sp.tile([16, 1], fp32, name="epsq")
            nc.vector.memset(epsq_sb, EPS)
            epsk_sb = sp.tile([16, 1], fp32, name="epsk")
            nc.vector.memset(epsk_sb, 64.0 * EPS)

            wv_b = wp.tile([128, KC * DIM], bf16, name="wv", tag="w")
            dma_pieces(wv_b, wv_d, 4, KC, DIM)
            wv_sb = [wv_b[:, k * DIM:(k + 1) * DIM] for k in range(KC)]

            qt_sb = []
            for k in range(KC):
                qt = qtp.tile([128, LC], bf16, name=f"qt{k}", tag="qt")
                nc.sync.dma_start(out=qt, in_=qT_d[k * 128:(k + 1) * 128, :])
                qt_sb.append(qt)

            wq_b = wp.tile([128, KC * DIM], bf16, name="wq", tag="w")
            dma_pieces(wq_b, wq_d, 4, KC, DIM)
            wq_sb = [wq_b[:, k * DIM:(k + 1) * DIM] for k in range(KC)]

            # -------- K projection + skT on the LOCAL kv half --------
            # m-outer: each khh[m] tile is complete after its sg loop and
            # its gather-blob staging DMA streams out while the rest of
            # the projection runs, so the AllGather launches right at
            # projection end.
            TOTA = KC * 128 * W
            TOTB = n_half * 128 * (VA_W + 32)
            off_sk = n_half * 128 * VA_W
            groups = [[2 * x, 2 * x + 1] for x in range(N_CORES // 2)]
            blobA_loc = dp.tile([TOTA], bf16, name="blobA_loc")
            blobA_g = dp.tile([2 * TOTA], bf16, name="blobA_g")
            khh_sb = [khb.tile([128, W], bf16, name=f"khh{m}", tag="khh",
                               bufs=KC) for m in range(KC)]
            skl_b = sp.tile([128, n_half * 16], fp32, name="skl")
            ssk = [po.tile([16, 512], fp32, name=f"ssk{sg}", tag="po")
                   for sg in range(n_hg)]
            pend = None
            for m in range(KC):
                for sg in range(n_hg):
                    wdt = min(512, W - sg * 512)
                    ps = pa.tile([128, 1024], fp32, name="proj_ps",
                                 tag="pa")
                    for k in range(KC):
                        nc.tensor.matmul(
                            ps[:, :wdt],
                            lhsT=wk_sb[k][:, m * 128:(m + 1) * 128],
                            rhs=kvt_sb[k][:, sg * 512:sg * 512 + wdt],
                            start=(k == 0), stop=(k == KC - 1))
                    if pend is not None:
                        pm, psg, pq, pwdt = pend
                        nc.tensor.matmul(
                            ssk[psg][:, :pwdt], lhsT=ind_sb[pm],
                            rhs=pq[:, :pwdt],
                            start=(pm == 0), stop=(pm == KC - 1))
                    nc.vector.tensor_scalar_mul(
                        khh_sb[m][:, sg * 512:sg * 512 + wdt], ps[:, :wdt],
                        kw_sb)
                    ksq = sqp.tile([128, 1024], bf16, name="sqt", tag="sq",
                                   bufs=6)
                    nc.scalar.activation(ksq[:, :wdt], ps[:, :wdt],
                                         AF.Square)
                    pend = (m, sg, ksq, wdt)
                nc.sync.dma_start(
                    out=blobA_loc[m * 128 * W:(m + 1) * 128 * W]
                    .rearrange("(p x) -> p x", p=128),
                    in_=khh_sb[m][:, :])
            pm, psg, pq, pwdt = pend
            nc.tensor.matmul(ssk[psg][:, :pwdt], lhsT=ind_sb[pm],
                             rhs=pq[:, :pwdt], start=False, stop=True)
            # 8*sqrt(mean+eps) = sqrt(sumsq + 64 eps); recip -> sk/8
            for sg in range(n_hg):
                wdt = min(512, W - sg * 512)
                skr = tp.tile([16, 512], fp32, name="skr", tag="small16",
                              bufs=2)
                nc.scalar.activation(skr[:, :wdt], ssk[sg][:, :wdt],
                                     AF.Sqrt, scale=1.0, bias=epsk_sb)
                for t in range(wdt // 128):
                    tpp = po.tile([128, 16], fp32, name="sktp", tag="po")
                    nc.tensor.transpose(tpp, skr[:, t * 128:(t + 1) * 128],
                                        eye_sb)
                    i = sg * 4 + t
                    nc.vector.reciprocal(out=skl_b[:, i * 16:(i + 1) * 16],
                                         in_=tpp)
            nc.gpsimd.collective_compute(
                "AllGather", mybir.AluOpType.bypass, replica_groups=groups,
                ins=[blobA_loc.opt()], outs=[blobA_g.opt()])

            # -------- V projection on the LOCAL kv half (ones-augmented) ----
            # per-chunk tiles: each chunk's gather-blob staging DMA streams
            # out during the rest of the projection.
            blobB_loc = dp.tile([TOTB], bf16, name="blobB_loc")
            blobB_g = dp.tile([2 * TOTB], bf16, name="blobB_g")
            for i in range(n_half):
                va = vb.tile([128, VA_W], bf16, name=f"val{i}", tag="val",
                             bufs=n_half)
                ones_cols = bass.AP(tensor=va.tensor, offset=va.offset + 64,
                                    ap=[list(va.ap[0]), [65, H], [1, 1]])
                nc.vector.memset(ones_cols, 1.0)
                ps = pa.tile([128, 1024], fp32, name="proj_ps", tag="pa")
                for k in range(KC):          # k outer: one LDW serves both jn
                    for jn in range(2):
                        nc.tensor.matmul(
                            ps[:, jn * 512:(jn + 1) * 512],
                            lhsT=kvt_sb[k][:, i * 128:(i + 1) * 128],
                            rhs=wv_sb[k][:, jn * 512:(jn + 1) * 512],
                            start=(k == 0), stop=(k == KC - 1))
                for jn in range(2):
                    dst = bass.AP(tensor=va.tensor,
                                  offset=va.offset + 65 * 8 * jn,
                                  ap=[list(va.ap[0]), [65, 8], [1, 64]])
                    nc.vector.tensor_copy(
                        dst, ps[:, jn * 512:(jn + 1) * 512]
                        .rearrange("p (h d) -> p h d", h=8))
                nc.sync.dma_start(
                    out=blobB_loc[i * 128 * VA_W:(i + 1) * 128 * VA_W]
                    .rearrange("(p x) -> p x", p=128),
                    in_=va[:, :])

            # -------- AllGather 2: va + sk (overlaps the Q projection) ----
            nc.sync.dma_start(
                out=blobB_loc[off_sk:TOTB]
                .rearrange("(l p x) -> p l x", l=n_half, p=128),
                in_=skl_b.bitcast(bf16).rearrange("p (l x) -> p l x",
                                                  l=n_half))
            nc.gpsimd.collective_compute(
                "AllGather", mybir.AluOpType.bypass, replica_groups=groups,
                ins=[blobB_loc.opt()], outs=[blobB_g.opt()])

            # ---------------- Q projection + q RMS stats ----------------
            # sumsq matmuls are emitted one (m, j) step late so the PE
            # queue head never blocks on the ACT Square.
            qh_sb = [qhp.tile([128, LC], bf16, name=f"qh{m}", tag="qh")
                     for m in range(KC)]
            sumsq_q = [po.tile([16, 512], fp32, name=f"ssq{j}", tag="po")
                       for j in range(2)]
            pend = None                      # (m, qsq_tile)
            for m in range(KC):
                ps = pa.tile([128, 1024], fp32, name="proj_ps", tag="pa")
                for k in range(KC):          # k outer: one LDW serves both j
                    for j in range(2):
                        nc.tensor.matmul(
                            ps[:, j * 512:(j + 1) * 512],
                            lhsT=wq_sb[k][:, m * 128:(m + 1) * 128],
                            rhs=qt_sb[k][:, j * 512:(j + 1) * 512],
                            start=(k == 0), stop=(k == KC - 1))
                if pend is not None:
                    pm, pq = pend
                    for j in range(2):
                        nc.tensor.matmul(
                            sumsq_q[j][:, :], lhsT=ind_sb[pm],
                            rhs=pq[:, j * 512:(j + 1) * 512],
                            start=(pm == 0), stop=(pm == KC - 1))
                nc.vector.tensor_scalar_mul(qh_sb[m][:, :], ps[:, :], qw_sb)
                qsq = sqp.tile([128, 1024], bf16, name="sqt", tag="sq",
                               bufs=6)
                nc.scalar.activation(qsq, ps, AF.Square)
                pend = (m, qsq)
            pm, pq = pend
            for j in range(2):
                nc.tensor.matmul(sumsq_q[j][:, :], lhsT=ind_sb[pm],
                                 rhs=pq[:, j * 512:(j + 1) * 512],
                                 start=False, stop=True)
            # sq = 1/sqrt(mean + eps), broadcast onto qhT via ind2 matmuls
            sq_sb = []
            for j in range(2):
                sqr = tp.tile([16, 512], fp32, name=f"sqr{j}", tag="small16",
                              bufs=2)
                nc.scalar.activation(sqr, sumsq_q[j][:, :], AF.Sqrt,
                                     scale=1.0 / HD, bias=epsq_sb)
                sqv = tp.tile([16, 512], bf16, name=f"sqv{j}",
                              tag="small16b", bufs=2)
                with nc.allow_low_precision(
                        reason="rsqrt scale applied to bf16 qh"):
                    nc.vector.reciprocal(out=sqv, in_=sqr)
                sq_sb.append(sqv)
            for m in range(KC):
                for j in range(2):
                    bc = po.tile([128, 512], fp32, name="qbc", tag="po")
                    nc.tensor.matmul(bc, lhsT=ind2_sb[m], rhs=sq_sb[j],
                                     start=True, stop=True)
                    nc.vector.tensor_mul(
                        qh_sb[m][:, j * 512:(j + 1) * 512],
                        qh_sb[m][:, j * 512:(j + 1) * 512], bc)

            # wo load: issued here so the DMA overlaps the attention phase
            # (the big tile reuses wk's pool slot, free once K-proj is done).
            wo_b = wp.tile([128, KC * DIM], bf16, name="wo", tag="w")
            nc.sync.dma_start(
                out=wo_b.rearrange("p (k c) -> p k c", k=KC),
                in_=wo_d.rearrange("(k p) c -> p k c", p=128))
            wo_sb = [wo_b[:, k * DIM:(k + 1) * DIM] for k in range(KC)]

            # -------- readback into canonical full-S tiles --------
            # per-(m, r) kh pieces and per-chunk va pieces so the
            # transfers spread over the DMA engines and stream in chunk
            # order just ahead of the attention pipeline's use.
            kh_sb = [khb.tile([128, 2 * W], bf16, name=f"kh{m}", tag="kh",
                              bufs=KC) for m in range(KC)]
            for m in range(KC):
                for r in range(2):
                    nc.sync.dma_start(
                        out=kh_sb[m][:, r * W:(r + 1) * W],
                        in_=blobA_g[r * TOTA + m * 128 * W:
                                    r * TOTA + (m + 1) * 128 * W]
                        .rearrange("(p x) -> p x", p=128))
            skT_b = sp.tile([128, n_sc * 16], fp32, name="skT")
            for r in range(2):
                cnt = n_half if r == 0 else n_sc - n_half
                nc.sync.dma_start(
                    out=skT_b.bitcast(bf16)[:, r * n_half * 32:
                                            (r * n_half + cnt) * 32]
                    .rearrange("p (l x) -> p l x", l=cnt),
                    in_=blobB_g[r * TOTB + off_sk:
                                r * TOTB + off_sk + cnt * 128 * 32]
                    .rearrange("(l p x) -> p l x", l=cnt, p=128))
            va_sb = []
            for i in range(n_sc):
                r, li = i // n_half, i % n_half
                va = vb.tile([128, VA_W], bf16, name=f"va{i}", tag="va",
                             bufs=n_sc)
                nc.sync.dma_start(
                    out=va[:, :],
                    in_=blobB_g[r * TOTB + li * 128 * VA_W:
                                r * TOTB + (li + 1) * 128 * VA_W]
                    .rearrange("(p x) -> p x", p=128))
                va_sb.append(va)

            # ---------------- attention (head pairs, pipelined) ----------
            # oT is normalized in two 4-pair groups: per pair the 4 pv
            # denominator rows (partition 64) are cast into one staging
            # tile and moved by ONE DMA into the packed group tile; one
            # batched DVE reciprocal per group (hidden behind attention /
            # the O-proj k<4 accumulation), then per (pair, j) one K=8
            # indicator matmul broadcast + one in-place DVE multiply.
            oT_sb = [qtp.tile([128, LC], bf16, name=f"oT{m}", tag="qt")
                     for m in range(KC)]
            den8 = [sp.tile([8, LC], bf16, name=f"den8_{g}")
                    for g in range(2)]
            rec8 = [sp.tile([8, LC], bf16, name=f"rec8_{g}")
                    for g in range(2)]

            def norm_apply(g):
                # broadcast 1/den and rescale oT for pairs 4g..4g+3
                for r in range(4):
                    pp = 4 * g + r
                    for j in range(2):
                        dbc = po.tile([128, 512], fp32, name="dbc",
                                      tag="po")
                        nc.tensor.matmul(
                            dbc, lhsT=ind2p8_sb[r],
                            rhs=rec8[g][:, j * 512:(j + 1) * 512],
                            start=True, stop=True)
                        nc.vector.tensor_mul(
                            oT_sb[pp][:, j * 512:(j + 1) * 512],
                            oT_sb[pp][:, j * 512:(j + 1) * 512], dbc)

            for p in range(KC):              # head pair (2p, 2p+1)
                hA, hB = 2 * p, 2 * p + 1
                pv = [po.tile([128, 512], fp32, name=f"pv{x}", tag="po")
                      for x in range(4)]     # A0 A1 B0 B1
                pending = None               # (exA, exB, first)
                for i in range(n_sc):
                    scA = pa.tile([128, 1024], fp32, name="scA", tag="pa")
                    scB = pa.tile([128, 1024], fp32, name="scB", tag="pa")
                    for j in range(2):
                        nc.tensor.matmul(
                            scA[:, j * 512:(j + 1) * 512],
                            lhsT=kh_sb[p][0:64, i * 128:(i + 1) * 128],
                            rhs=qh_sb[p][0:64, j * 512:(j + 1) * 512],
                            start=True, stop=True, tile_position=(0, 0))
                        nc.tensor.matmul(
                            scB[:, j * 512:(j + 1) * 512],
                            lhsT=kh_sb[p][64:128, i * 128:(i + 1) * 128],
                            rhs=qh_sb[p][64:128, j * 512:(j + 1) * 512],
                            start=True, stop=True, tile_position=(64, 0))
                    if pending is not None:
                        exA, exB, first = pending
                        for j in range(2):
                            nc.tensor.matmul(
                                pv[j][:65, :],
                                lhsT=va_sb[i - 1][:, hA * 65:(hA + 1) * 65],
                                rhs=exA[:, j * 512:(j + 1) * 512],
                                start=first, stop=False)
                        for j in range(2):
                            nc.tensor.matmul(
                                pv[2 + j][:65, :],
                                lhsT=va_sb[i - 1][:, hB * 65:(hB + 1) * 65],
                                rhs=exB[:, j * 512:(j + 1) * 512],
                                start=first, stop=False)
                    exA = sqp.tile([128, 1024], bf16, name="exA", tag="sq",
                                   bufs=6)
                    exB = sqp.tile([128, 1024], bf16, name="exB", tag="sq",
                                   bufs=6)
                    nc.scalar.activation(
                        exA, scA, AF.Exp,
                        scale=skT_b[:, i * 16 + hA:i * 16 + hA + 1],
                        bias=mask_sb[:, i:i + 1])
                    nc.scalar.activation(
                        exB, scB, AF.Exp,
                        scale=skT_b[:, i * 16 + hB:i * 16 + hB + 1],
                        bias=mask_sb[:, i:i + 1])
                    pending = (exA, exB, i == 0)
                exA, exB, first = pending
                for j in range(2):
                    nc.tensor.matmul(
                        pv[j][:65, :],
                        lhsT=va_sb[n_sc - 1][:, hA * 65:(hA + 1) * 65],
                        rhs=exA[:, j * 512:(j + 1) * 512],
                        start=first, stop=True)
                for j in range(2):
                    nc.tensor.matmul(
                        pv[2 + j][:65, :],
                        lhsT=va_sb[n_sc - 1][:, hB * 65:(hB + 1) * 65],
                        rhs=exB[:, j * 512:(j + 1) * 512],
                        start=first, stop=True)
                # stash unnormalized o^T; cast the 4 den rows into one
                # staging tile and move them with ONE DMA into the packed
                # [8, LC] group tile (rows 2r, 2r+1).
                g, r = p // 4, p % 4
                dstage = tp.tile([128, 2048], bf16, name="dstage",
                                 tag="rec", bufs=2)
                for x, (hh, j) in enumerate(((hA, 0), (hA, 1),
                                             (hB, 0), (hB, 1))):
                    poff = (hh % 2) * 64
                    nc.vector.tensor_copy(
                        oT_sb[p][poff:poff + 64, j * 512:(j + 1) * 512],
                        pv[x][0:64, :])
                    nc.vector.tensor_copy(
                        dstage[64:65, x * 512:(x + 1) * 512],
                        pv[x][64:65, :])
                nc.sync.dma_start(out=den8[g][2 * r:2 * r + 2, :],
                                  in_=dstage[64:65, :])
                if p == 4:
                    # group-0 reciprocal on DVE, overlapped with pair 5
                    with nc.allow_low_precision(
                            reason="1/den scale applied to bf16 oT"):
                        nc.vector.reciprocal(out=rec8[0], in_=den8[0])
                if p == 5:
                    norm_apply(0)

            # ---------------- output projection ----------------
            # k-outer with 8 parallel PSUM accumulators.  The group-1
            # normalization is interleaved: a few k<3 groups of the lc0-3
            # accumulators run while the reciprocal drains, then the
            # broadcast+rescale, then everything else.  PSUM->SBUF copies
            # alternate scalar (idle post-attention) and vector; the out
            # DMA is one instruction per 512-col half.
            with nc.allow_low_precision(
                    reason="1/den scale applied to bf16 oT"):
                nc.vector.reciprocal(out=rec8[1], in_=den8[1])
            oacc = []
            for t in range(2):
                pt = pa.tile([128, 1024], fp32, name=f"oacc{t}", tag="pa")
                oacc.append(pt[:, 0:512])
                oacc.append(pt[:, 512:1024])

            def oproj_mm(jn, k, lc):
                nc.tensor.matmul(
                    oacc[lc], lhsT=oT_sb[k][:, lc * 128:(lc + 1) * 128],
                    rhs=wo_sb[k][:, jn * 512:(jn + 1) * 512],
                    start=(k == 0), stop=(k == KC - 1))

            for k in range(3):               # covers the reciprocal latency
                for lc in range(4):
                    oproj_mm(0, k, lc)
            norm_apply(1)
            for k in range(3, KC):
                for lc in range(4):
                    oproj_mm(0, k, lc)
            for t in range(4):
                oacc.append(po.tile([128, 512], fp32, name=f"oaccb{t}",
                                    tag="po"))
            for k in range(KC):
                for lc in range(4, KC):
                    oproj_mm(0, k, lc)
            for jn in range(2):
                if jn == 1:
                    for k in range(KC):
                        for lc in range(KC):
                            oproj_mm(1, k, lc)
                for lc in range(KC):
                    osb = tp.tile([128, 512], bf16, name="osb", tag="osb",
                                  bufs=4)
                    if lc % 2 == 0:
                        nc.scalar.copy(out=osb, in_=oacc[lc])
                    else:
                        nc.vector.tensor_copy(osb, oacc[lc])
                    nc.sync.dma_start(
                        out=out_d[lc * 128:(lc + 1) * 128,
                                  jn * 512:(jn + 1) * 512],
                        in_=osb)
    nc.compile()
    return nc


def kernel(**inputs):
    q = np.asarray(inputs["q"], dtype=np.float32)
    kv = np.asarray(inputs["kv"], dtype=np.float32)
    seqlens = np.asarray(inputs["x_seqlens"], dtype=np.int32)
    Wq = np.asarray(inputs["Wq"], dtype=np.float32)
    Wk = np.asarray(inputs["Wk"], dtype=np.float32)
    Wv = np.asarray(inputs["Wv"], dtype=np.float32)
    Wo = np.asarray(inputs["Wo"], dtype=np.float32)
    qnw = np.asarray(inputs["q_norm_w"], dtype=np.float32)
    knw = np.asarray(inputs["k_norm_w"], dtype=np.float32)

    n_sc = max(1, int(-(-int(seqlens.max()) // 128)))
    if n_sc not in _CACHE:
        _CACHE[n_sc] = _build(n_sc)
    nc = _CACHE[n_sc]

    wq_b = np.ascontiguousarray(Wq).astype(BF16)
    wk_b = np.ascontiguousarray(Wk).astype(BF16)
    wv_b = np.ascontiguousarray(Wv).astype(BF16)
    wo_b = np.ascontiguousarray(Wo).astype(BF16)
    ind = np.zeros((KC, 128, 16), np.float32)
    ind2 = np.zeros((KC, 16, 128), np.float32)
    p = np.arange(128)
    for c in range(KC):
        ind[c, p, 2 * c + p // 64] = 1.0
        ind2[c, 2 * c + p // 64, p] = 1.0
    cf32 = np.zeros((128, 34), np.float32)
    cf32[:, 16] = np.tile(qnw, 2)
    cf32[:, 17] = np.tile(knw, 2)
    cf32[0:16, 18:34] = np.eye(16, dtype=np.float32)
    cbf = np.zeros((128, 1664), np.float32)
    for k in range(KC):
        cbf[:, k * 16:(k + 1) * 16] = ind[k]
    for m in range(KC):
        cbf[0:16, 128 + m * 128:128 + (m + 1) * 128] = ind2[m]
    for r in range(4):
        cbf[2 * r, 1152 + r * 128:1152 + r * 128 + 64] = 1.0
        cbf[2 * r + 1, 1152 + r * 128 + 64:1152 + (r + 1) * 128] = 1.0
    cbf = cbf.astype(BF16)

    in_maps = []
    for c in range(N_CORES):
        b, half = c // 2, c % 2
        qT = np.ascontiguousarray(
            q[b, half * LC:(half + 1) * LC, :].T).astype(BF16)
        n_half = (n_sc + 1) // 2
        Wl = n_half * 128
        kvT = np.ascontiguousarray(
            kv[b].T[:, half * Wl:(half + 1) * Wl]).astype(BF16)
        sl = int(seqlens[b])
        mask = np.where(np.arange(S) < sl, 0.0, MASK_BIAS).astype(np.float32)
        cfc = cf32.copy()
        cfc[:, 0:16] = np.ascontiguousarray(mask.reshape(16, 128).T)
        in_maps.append({
            "qT": qT, "kvT": kvT, "wq": wq_b, "wk": wk_b, "wv": wv_b,
            "wo": wo_b, "cf32": cfc, "cbf": cbf,
        })

    res = run_bass_kernel_spmd(nc, in_maps, list(range(N_CORES)),
                               trace=TRACE)
    LAST_RESULT["exec_time_ns"] = res.exec_time_ns
    LAST_RESULT["profile"] = res.profile_json

    out = np.empty((B, L, DIM), np.float32)
    for c in range(N_CORES):
        b, half = c // 2, c % 2
        out[b, half * LC:(half + 1) * LC, :] = \
            res.results[c]["out"].astype(np.float32)
    return out
